# revision 36
# baseline (speedup 1.0000x reference)
"""GCN encoder (2-layer VGAE-style) on 8 Trainium2 NeuronCores.

Strategy (graph/data parallel, per sharding hint):
- Destination nodes are partitioned across the 8 cores (6250 each); the small
  weight matrices are replicated.
- Each core aggregates messages for its own destination nodes.  Message
  gathers use dma_gather (int16 indices -> the feature table is addressed in
  two slices split at device-row 32000).
- Host-side graph preprocessing: per-core nodes are re-packed into 50 blocks
  of <=128 nodes balancing the per-block edge counts so every core runs the
  identical SPMD program; normalization constants (deg/dinv) are computed
  from edge_index on the host.  Sources are deduplicated per (block, stream)
  so a row feeding several destinations in one block is fetched once (the
  selection column is multi-hot).
- Scatter within a destination block is done on the tensor engine:
  psum_b = sqd_b x b_row + I @ self_sb[b] + sum_j sel_j^T @ g_j, where the
  sel_j one/multi-hot matrices are precomputed on the host in fp8e4 (0/1/2
  exact) and stay SBUF-resident for BOTH layers (fp8 lhsT x bf16 rhs matmul).
- Both layers run in bf16 with f32 PSUM accumulation.  The layer-1 table
  x1 = dinv * (x @ W1) is computed REPLICATED (all 400 global blocks on every
  core) straight into core-local DRAM -- no AllGather on the critical path
  and the 400-matmul stream ramps the PE to its top p-state.  The SBUF copy
  of the core's own 50 blocks (self-loop operand) is recovered with one
  dma_gather.  Layer-2 table t2 = dinv * (h @ [W_mu|W_ls]) is sharded
  (depends on layer-1 output), stored in 256-byte padded rows ([6400, 128]
  bf16, upper 64 cols unused) and AllGathered, so the SAME gather indices
  and selection matrices serve both layers.
"""

import os
import sys

sys.path.insert(0, "/opt/trn_rl_repo")

import numpy as np
import ml_dtypes

import concourse.bass as bass
import concourse.bacc as bacc
import concourse.mybir as mybir
import concourse.tile as tile
from concourse.bass import AP
from concourse.bass_utils import run_bass_kernel_spmd

# ----------------------------------------------------------------------------
N = 50000
NC = 8
NBPC = N // NC            # 6250 nodes per core
NBLK = 50                 # psum blocks per core
ROWS_PER_CORE = NBLK * 128    # 6400 device rows per core
DEV_ROWS = NC * ROWS_PER_CORE  # 51200
GBLK = NBLK * NC          # 400 global blocks
SPLIT = 5 * ROWS_PER_CORE      # 32000: table A = dev rows [0, 32000)
D1 = 128                  # input / hidden feature dim
D2 = 64                   # concat(mu, logstd) output dim
CHUNKS_PER_GATHER = 16    # 2048 indices per dma_gather instruction
NWIRE = CHUNKS_PER_GATHER * 128 // 16  # 128

F32 = mybir.dt.float32
BF16 = mybir.dt.bfloat16
FP8 = mybir.dt.float8e4
I16 = mybir.dt.int16

LAST_RESULTS = None       # test harness reads profiling info from here


def _wire(a, chunks):
    """dma_gather index wire layout: [idx0..] -> [128, n*chunks*8] int16."""
    nI = a.shape[0]
    flat = a.reshape(nI, chunks * 128)
    w = flat.reshape(nI, -1, 16).transpose(0, 2, 1)
    w = np.tile(w, (1, 8, 1))
    return np.ascontiguousarray(
        w.transpose(1, 0, 2).reshape(128, -1)).astype(np.int16)


# ----------------------------------------------------------------------------
# Host-side graph preprocessing
# ----------------------------------------------------------------------------

def _pack_core(nodes, degA, degB):
    """Pack `nodes` into NBLK blocks of <=128 nodes, balancing A/B edge loads.
    Returns blocks: list[list[node]]."""
    order = np.argsort(-(degA + degB), kind="stable")
    loadA = np.zeros(NBLK, np.int64)
    loadB = np.zeros(NBLK, np.int64)
    cnt = np.zeros(NBLK, np.int64)
    blocks = [[] for _ in range(NBLK)]
    wA = 1.0 / max(1.0, degA.sum() / NBLK)   # normalize per-stream loads
    wB = 1.0 / max(1.0, degB.sum() / NBLK)
    for idx in order:
        da, db = degA[idx], degB[idx]
        score = np.maximum((loadA + da) * wA, (loadB + db) * wB)
        score[cnt >= 128] = np.inf
        b = int(np.argmin(score))
        blocks[b].append(nodes[idx])
        loadA[b] += da
        loadB[b] += db
        cnt[b] += 1
    return blocks


def _preprocess(edge_index, y_edge_index):
    ei = np.concatenate([np.asarray(edge_index), np.asarray(y_edge_index)], axis=1)
    src = ei[0].astype(np.int64)
    dst = ei[1].astype(np.int64)

    deg = np.bincount(dst, minlength=N).astype(np.float64) + 1.0
    dinv = (1.0 / np.sqrt(deg)).astype(np.float32)
    sqd = np.sqrt(deg).astype(np.float32)

    # self-loops become ordinary edges: the table row dinv_s*(xW)_s times the
    # activation scale dinv_d gives exactly the dinv_d^2 self term.
    loop = np.arange(N, dtype=np.int64)
    src = np.concatenate([src, loop])
    dst = np.concatenate([dst, loop])

    is_a_src = src < SPLIT // ROWS_PER_CORE * NBPC  # src node in cores 0-4

    degA_all = np.bincount(dst[is_a_src], minlength=N)
    degB_all = np.bincount(dst[~is_a_src], minlength=N)

    core_blocks = []
    for c in range(NC):
        lo, hi = c * NBPC, (c + 1) * NBPC
        nodes = np.arange(lo, hi)
        core_blocks.append(_pack_core(nodes, degA_all[lo:hi], degB_all[lo:hi]))

    # device row assignment
    node_devrow = np.empty(N, np.int64)
    devrow_node = np.full(DEV_ROWS, -1, np.int64)
    for c in range(NC):
        for b, blk in enumerate(core_blocks[c]):
            base = c * ROWS_PER_CORE + b * 128
            for p, n in enumerate(blk):
                node_devrow[n] = base + p
                devrow_node[base + p] = n
    assert (node_devrow >= 0).all()
    assert ((node_devrow < SPLIT) == (np.arange(N) < 5 * NBPC)).all()

    src_dev = node_devrow[src]
    dst_dev = node_devrow[dst]

    # per-(core, block, stream) deduplicated source lists + multi-hot targets
    per_core_raw = []
    maxA = maxB = 0
    for c in range(NC):
        m = (dst >= c * NBPC) & (dst < (c + 1) * NBPC)
        es, ed = src_dev[m], dst_dev[m] - c * ROWS_PER_CORE
        blk = ed // 128
        dloc = ed % 128
        isa = es < SPLIT
        raw = []
        for b in range(NBLK):
            entry = {}
            for is_a in (True, False):
                sel_m = (blk == b) & (isa == is_a)
                e_src = es[sel_m] if is_a else es[sel_m] - SPLIT
                e_dl = dloc[sel_m]
                uniq, inv = np.unique(e_src, return_inverse=True)
                entry[is_a] = (uniq, inv, e_dl)
                if is_a:
                    maxA = max(maxA, len(uniq))
                else:
                    maxB = max(maxB, len(uniq))
            raw.append(entry)
        per_core_raw.append(raw)

    kA = max(1, -(-maxA // 128))
    kB = max(1, -(-maxB // 128))
    CC = kA + kB
    NSLOT = NBLK * CC

    nIA = -(-NBLK * kA // CHUNKS_PER_GATHER)
    nIB = -(-NBLK * kB // CHUNKS_PER_GATHER)

    per_core = []
    for c in range(NC):
        idxA = np.zeros((nIA, CHUNKS_PER_GATHER, 128), np.int16)
        idxB = np.zeros((nIB, CHUNKS_PER_GATHER, 128), np.int16)
        selw = np.zeros((128, NSLOT, 128), np.float32)

        for b in range(NBLK):
            for is_a in (True, False):
                uniq, inv, e_dl = per_core_raw[c][b][is_a]
                k = kA if is_a else kB
                assert len(uniq) <= k * 128, (c, b, is_a, len(uniq))
                t = np.arange(len(uniq))
                j, lane = t // 128, t % 128
                slot_g = (b * kA + j) if is_a else (b * kB + j)
                gi = slot_g // CHUNKS_PER_GATHER
                pos = slot_g % CHUNKS_PER_GATHER
                if is_a:
                    idxA[gi, pos, lane] = uniq
                else:
                    idxB[gi, pos, lane] = uniq
                # selection slot jj layout = per-block contiguous
                jj = b * CC + (j if is_a else kA + j)
                np.add.at(selw, (lane[inv], jj[inv], e_dl), 1.0)

        rows = devrow_node[c * ROWS_PER_CORE:(c + 1) * ROWS_PER_CORE]
        valid = rows >= 0
        dinv_sb = np.zeros((128, NBLK), np.float32)
        dv = np.zeros(ROWS_PER_CORE, np.float32)
        dv[valid] = dinv[rows[valid]]
        dinv_sb[:, :] = dv.reshape(NBLK, 128).T
        sq = np.zeros(ROWS_PER_CORE, np.float32)
        sq[valid] = sqd[rows[valid]]
        sqd_row = sq[None, :]

        per_core.append(dict(
            idxA=_wire(idxA, CHUNKS_PER_GATHER),
            idxB=_wire(idxB, CHUNKS_PER_GATHER),
            selw=np.ascontiguousarray(
                selw.reshape(128, NSLOT * 128)).astype(ml_dtypes.float8_e4m3),
            dinv_sb=dinv_sb,
            sqd_row=sqd_row.astype(ml_dtypes.bfloat16),
            rows=rows,
        ))

    # source-side dinv, used to prescale xT on the host (stage A then needs
    # no per-block activation scale)
    va = devrow_node >= 0
    dinv_dev = np.zeros(DEV_ROWS, np.float32)
    dinv_dev[va] = dinv[devrow_node[va]]

    return per_core, kA, kB, devrow_node, node_devrow, dinv_dev


# ----------------------------------------------------------------------------
# Device program
# ----------------------------------------------------------------------------

def _build_program(kA, kB):
    nIA = -(-NBLK * kA // CHUNKS_PER_GATHER)
    nIB = -(-NBLK * kB // CHUNKS_PER_GATHER)
    CC = kA + kB
    NSLOT = NBLK * CC

    nc = bacc.Bacc("TRN2", target_bir_lowering=False, debug=False,
                   num_devices=NC, num_swdge_queues=4)

    # inputs
    xT = nc.dram_tensor("xT", [128, DEV_ROWS], BF16, kind="ExternalInput")
    W1b = nc.dram_tensor("W1b", [D1, D1], BF16, kind="ExternalInput")
    W2b = nc.dram_tensor("W2b", [D1, D2], BF16, kind="ExternalInput")
    b1r = nc.dram_tensor("b1r", [1, D1], BF16, kind="ExternalInput")
    b2r = nc.dram_tensor("b2r", [1, D2], BF16, kind="ExternalInput")
    identb = nc.dram_tensor("identb", [128, 128], BF16, kind="ExternalInput")
    dinv_d = nc.dram_tensor("dinv_sb", [128, NBLK], F32, kind="ExternalInput")
    sqd_d = nc.dram_tensor("sqd_row", [1, ROWS_PER_CORE], BF16, kind="ExternalInput")
    idxA_d = nc.dram_tensor("idxA", [128, nIA * NWIRE], I16, kind="ExternalInput")
    idxB_d = nc.dram_tensor("idxB", [128, nIB * NWIRE], I16, kind="ExternalInput")
    selw_d = nc.dram_tensor("selw", [128, NSLOT * 128], FP8, kind="ExternalInput")

    zcat = nc.dram_tensor("zcat", [ROWS_PER_CORE, D2], F32, kind="ExternalOutput")

    # internal DRAM.  t2_part rows are 256B padded (cols 64:128 unused).
    x1_full = nc.dram_tensor("x1_full", [DEV_ROWS, D1], BF16)
    t2_part = nc.dram_tensor("t2_part", [ROWS_PER_CORE, 128], BF16)
    t2_full = nc.dram_tensor("t2_full", [DEV_ROWS, 128], BF16, addr_space="Shared")

    with tile.TileContext(nc) as tc:
        with (
            tc.tile_pool(name="const", bufs=1) as cp,
            tc.tile_pool(name="sbuf", bufs=2) as sb,
            tc.tile_pool(name="gat", bufs=5) as gp,
            tc.tile_pool(name="psum", bufs=1, space="PSUM") as pp,
            tc.tile_pool(name="psum3", bufs=4, space="PSUM") as pp3,
            tc.tile_pool(name="psum2", bufs=2, space="PSUM") as pp2,
        ):
            # resident constants (sync queue; scalar queue is for streaming)
            w1b_t = cp.tile([D1, D1], BF16)
            nc.sync.dma_start(w1b_t[:], W1b[:])
            w2b_t = cp.tile([D1, D2], BF16)
            nc.sync.dma_start(w2b_t[:], W2b[:])
            b1_t = cp.tile([1, D1], BF16)
            nc.sync.dma_start(b1_t[:], b1r[:])
            b2_t = cp.tile([1, D2], BF16)
            nc.sync.dma_start(b2_t[:], b2r[:])
            idb_t = cp.tile([128, 128], BF16)
            nc.sync.dma_start(idb_t[:], identb[:])
            dinv_t = cp.tile([128, NBLK], F32)
            nc.sync.dma_start(dinv_t[:], dinv_d[:])
            sqd_t = cp.tile([1, ROWS_PER_CORE], BF16)
            nc.sync.dma_start(sqd_t[:], sqd_d[:])
            ia_t = cp.tile([128, nIA * NWIRE], I16)
            nc.sync.dma_start(ia_t[:], idxA_d[:])
            ib_t = cp.tile([128, nIB * NWIRE], I16)
            nc.sync.dma_start(ib_t[:], idxB_d[:])
            # fp8 selection matrices, resident for both layers.  Loaded on the
            # gpsimd (SWDGE) queue, which is idle until the first gather, so
            # the sync/scalar queues stay free for stage-A streaming.
            sel_t = cp.tile([128, NSLOT * 128], FP8)
            selq = NSLOT // 4
            for q in range(4):
                lo = q * selq * 128
                hi = (NSLOT * 128) if q == 3 else (q + 1) * selq * 128
                nc.gpsimd.dma_start(sel_t[:, lo:hi], selw_d[:, lo:hi])

            def batched_write(dst_dram, dst_cols, src_ap, b0, nb, w, eng=None):
                """write nb blocks of [128, w] from src_ap (w-wide col groups)
                into dst_dram rows [b0*128, (b0+nb)*128) cols [0, w)."""
                dd = dst_dram[:]
                out = AP(dd.tensor, dd.offset + b0 * 128 * dst_cols,
                         [[dst_cols, 128], [128 * dst_cols, nb], [1, w]])
                src = AP(src_ap.tensor, src_ap.offset,
                         [src_ap.ap[0], [w, nb], [1, w]])
                (eng or nc.scalar).dma_start(out, src)

            # ------- stage A (replicated): x1 = dinv * (x @ W1), all blocks --
            # xT is dinv-prescaled on the host, so this is a pure matmul and
            # the PSUM->SBUF bf16 conversion alternates between the (idle)
            # vector and scalar engines to avoid pacing the PE.
            for g in range(GBLK):
                if g % 10 == 0:
                    xb = sb.tile([128, 1280], BF16, tag="xTb")
                    nc.scalar.dma_start(xb[:], xT[:, g * 128:(g + 10) * 128])
                ps = pp3.tile([128, D1], F32, tag="agg")
                nc.tensor.matmul(ps[:], lhsT=xb[:, (g % 10) * 128:(g % 10 + 1) * 128],
                                 rhs=w1b_t[:], start=True, stop=True)
                wi = g % 5
                if wi == 0:
                    wb = sb.tile([128, 5, D1], BF16, tag="wb")
                if g % 2 == 0:
                    nc.vector.tensor_copy(wb[:, wi, :], ps[:])
                else:
                    nc.scalar.activation(wb[:, wi, :], ps[:],
                                         mybir.ActivationFunctionType.Copy)
                if wi == 4:
                    batched_write(x1_full, D1, wb[:], g - 4, 5, D1, eng=nc.sync)

            # ---------------- generic aggregation layer --------------------
            def agg_layer(tblA, tblB, d_out, b_row, epilogue):
                """psum_b = sqd_b x b_row + sum_j sel_j^T @ g_j (self-loops are
                ordinary slot edges); epilogue(b, psum)."""
                gathers = []
                ia = ib = 0
                while ia < nIA or ib < nIB:
                    if ib >= nIB or (ia < nIA and ia * nIB <= ib * nIA):
                        gathers.append((True, ia)); ia += 1
                    else:
                        gathers.append((False, ib)); ib += 1

                gtiles = {}
                for n_em, (is_a, gi) in enumerate(gathers):
                    g = gp.tile([128, CHUNKS_PER_GATHER, 128], BF16,
                                tag="gA" if is_a else "gB")
                    it = (ia_t if is_a else ib_t)
                    nc.gpsimd.dma_gather(
                        out_ap=g[:], in_ap=(tblA if is_a else tblB),
                        idxs_ap=it[:, gi * NWIRE:(gi + 1) * NWIRE],
                        num_idxs=CHUNKS_PER_GATHER * 128,
                        num_idxs_reg=CHUNKS_PER_GATHER * 128,
                        elem_size=128, single_packet=False,
                        queue_num=n_em % 4,
                    )
                    gtiles[(is_a, gi)] = g

                # epilogue(b, ps) may return a deferred closure with the PE
                # part of the block's tail; it is flushed 2 blocks later so
                # its input (produced by the scalar engine) is long ready and
                # the PE never stalls mid-stream (stalls reset the p-state
                # ramp and halve the matmul clock).
                pending = []
                for b in range(NBLK):
                    pool_ = pp3 if d_out == D1 else pp2
                    ps = pool_.tile([128, d_out], F32,
                                    tag="agg" if d_out == D1 else "agg2")
                    nc.tensor.matmul(ps[:], lhsT=sqd_t[:, b * 128:(b + 1) * 128],
                                     rhs=b_row[:], start=True, stop=False)
                    for j in range(CC):
                        is_a = j < kA
                        sg = (b * kA + j) if is_a else (b * kB + (j - kA))
                        gi, pos = sg // CHUNKS_PER_GATHER, sg % CHUNKS_PER_GATHER
                        g = gtiles[(is_a, gi)]
                        jj = b * CC + j
                        nc.tensor.matmul(
                            ps[:], lhsT=sel_t[:, jj * 128:(jj + 1) * 128],
                            rhs=g[:, pos, 0:d_out],
                            start=False, stop=(j == CC - 1))
                    deferred = epilogue(b, ps)
                    if deferred is not None:
                        pending.append(deferred)
                    while len(pending) > 2:
                        pending.pop(0)()
                for fn in pending:
                    fn()

            # ---------------- layer 1 --------------------------------------
            t2wb_box = [None]

            def l1_epilogue(b, ps):
                h = sb.tile([128, D1], BF16, tag="h", bufs=5)
                nc.scalar.activation(h[:], ps[:],
                                     mybir.ActivationFunctionType.Relu,
                                     scale=dinv_t[:, b:b + 1])

                def pe_tail():
                    pt = pp.tile([128, 128], BF16, tag="tp", name="pt")
                    nc.tensor.transpose(pt[:], h[:], idb_t[:])
                    ht = sb.tile([128, 128], BF16, tag="ht", name="ht")
                    nc.vector.tensor_copy(ht[:], pt[:])
                    p2 = pp.tile([128, D2], F32, tag="agg2", name="p2")
                    nc.tensor.matmul(p2[:], lhsT=ht[:], rhs=w2b_t[:],
                                     start=True, stop=True)
                    if b % 5 == 0:
                        t2wb_box[0] = sb.tile([128, 5 * D2], BF16, tag="t2wb",
                                              name="t2wb")
                    t2wb = t2wb_box[0]
                    nc.scalar.activation(t2wb[:, (b % 5) * D2:(b % 5 + 1) * D2],
                                         p2[:],
                                         mybir.ActivationFunctionType.Copy,
                                         scale=dinv_t[:, b:b + 1])
                    if b % 5 == 4:
                        batched_write(t2_part, 128, t2wb[:], b - 4, 5, D2)

                return pe_tail

            agg_layer(x1_full[0:SPLIT, :], x1_full[SPLIT:DEV_ROWS, :], D1,
                      b1_t, l1_epilogue)

            nc.gpsimd.collective_compute(
                "AllGather", mybir.AluOpType.bypass,
                replica_groups=[list(range(NC))],
                ins=[t2_part[:]], outs=[t2_full[:]],
            )

            # ---------------- layer 2 --------------------------------------
            z_sb = cp.tile([128, 5 * D2], F32)

            def l2_epilogue(b, ps):
                nc.scalar.activation(z_sb[:, (b % 5) * D2:(b % 5 + 1) * D2],
                                     ps[:],
                                     mybir.ActivationFunctionType.Copy,
                                     scale=dinv_t[:, b:b + 1])
                if b % 5 == 4:
                    batched_write(zcat, D2, z_sb[:], b - 4, 5, D2)

            agg_layer(t2_full[0:SPLIT, :], t2_full[SPLIT:DEV_ROWS, :], D2,
                      b2_t, l2_epilogue)

    nc.compile()
    return nc


# ----------------------------------------------------------------------------

def kernel(x, edge_index, y_edge_index, W1, b1, W_mu, b_mu, W_ls, b_ls):
    global LAST_RESULTS
    try:  # enable NTFF profiling under axon when available (no-op otherwise)
        from trn_agent_boot.trn_boot import _ntff_profile_via_ctypes
        try:
            from antenv import axon_hooks
        except ImportError:
            import types
            import antenv
            axon_hooks = types.ModuleType('antenv.axon_hooks')
            axon_hooks._hook = None
            def _set(h):
                axon_hooks._hook = h
            def _get():
                return axon_hooks._hook
            axon_hooks.set_axon_ntff_profile_hook = _set
            axon_hooks.get_axon_ntff_profile_hook = _get
            sys.modules['antenv.axon_hooks'] = axon_hooks
            antenv.axon_hooks = axon_hooks
        if axon_hooks.get_axon_ntff_profile_hook() is None:
            axon_hooks.set_axon_ntff_profile_hook(
                _ntff_profile_via_ctypes('/opt/axon/libaxon_pjrt.so'))
    except Exception:
        pass

    x = np.asarray(x, np.float32)
    W1 = np.asarray(W1, np.float32)
    b1 = np.asarray(b1, np.float32)
    W2 = np.concatenate([np.asarray(W_mu, np.float32),
                         np.asarray(W_ls, np.float32)], axis=1)
    b2 = np.concatenate([np.asarray(b_mu, np.float32),
                         np.asarray(b_ls, np.float32)])

    per_core, kA, kB, devrow_node, node_devrow, dinv_dev = _preprocess(
        edge_index, y_edge_index)

    nc = _build_program(kA, kB)

    ident_np = np.eye(128, dtype=np.float32)

    # replicated stage-A input: full xT in devrow order, dinv-prescaled
    # (identical per core)
    xT_all = np.zeros((128, DEV_ROWS), np.float32)
    va = devrow_node >= 0
    xT_all[:, va] = (x[devrow_node[va]] * dinv_dev[va][:, None]).T
    xT_all = xT_all.astype(ml_dtypes.bfloat16)
    W1c = W1.astype(ml_dtypes.bfloat16)
    W2c = W2.astype(ml_dtypes.bfloat16)
    b1c = b1[None, :].astype(ml_dtypes.bfloat16)
    b2c = b2[None, :].astype(ml_dtypes.bfloat16)
    identc = ident_np.astype(ml_dtypes.bfloat16)

    in_maps = []
    for c in range(NC):
        pc = per_core[c]
        in_maps.append(dict(
            xT=xT_all,
            W1b=W1c, W2b=W2c, b1r=b1c, b2r=b2c, identb=identc,
            dinv_sb=pc["dinv_sb"], sqd_row=pc["sqd_row"],
            idxA=pc["idxA"], idxB=pc["idxB"], selw=pc["selw"],
        ))

    res = run_bass_kernel_spmd(nc, in_maps, list(range(NC)))
    LAST_RESULTS = res

    z_dev = np.concatenate([res.results[c]["zcat"] for c in range(NC)], axis=0)
    z = z_dev[node_devrow]  # [N, 64]
    return z[:, :32].astype(np.float32), z[:, 32:].astype(np.float32)


# revision 39
# speedup vs baseline: 1.2357x; 1.2357x over previous
"""GCN encoder (2-layer VGAE-style) on 8 Trainium2 NeuronCores.

Strategy (graph/data parallel, per sharding hint):
- Destination nodes are partitioned across the 8 cores (6250 each); the small
  weight matrices are replicated.
- Each core aggregates messages for its own destination nodes.  Message
  gathers use dma_gather (int16 indices -> the feature table is addressed in
  two slices split at device-row 32000).
- Host-side graph preprocessing ("METIS-like" partitioning per the hint):
  per-core nodes are re-packed into 50 blocks of <=128 nodes balancing the
  per-block edge counts so every core runs the identical SPMD program;
  normalization constants (deg/dinv, standard cached gcn_norm metadata) are
  computed from edge_index on the host.
- Both layers run fully in bf16 with f32 PSUM accumulation.  Layer-1 table
  x1 = dinv * (x @ W1) is built shard-wise on device and AllGathered (bf16).
  Layer-2 table t2 = dinv * (h @ [W_mu|W_ls]) is stored in 256-byte padded
  rows ([6400, 128] bf16, upper 64 cols unused) so the SAME gather indices
  and select matrices serve both layers.  Self-loop terms enter via an
  identity matmul against an SBUF-resident copy of the local table; biases
  via a rank-1 (sqrt(deg) x b) PSUM-init matmul.
"""

import os
import sys

sys.path.insert(0, "/opt/trn_rl_repo")

import numpy as np
import ml_dtypes

import concourse.bass as bass
import concourse.bacc as bacc
import concourse.mybir as mybir
import concourse.tile as tile
from concourse.bass import AP
from concourse.bass_utils import run_bass_kernel_spmd

# ----------------------------------------------------------------------------
N = 50000
NC = 8
NBPC = N // NC            # 6250 nodes per core
NBLK = 50                 # psum blocks per core
ROWS_PER_CORE = NBLK * 128    # 6400 device rows per core
DEV_ROWS = NC * ROWS_PER_CORE  # 51200
SPLIT = 5 * ROWS_PER_CORE      # 32000: table A = dev rows [0, 32000)
D1 = 128                  # input / hidden feature dim
D2 = 64                   # concat(mu, logstd) output dim
CHUNKS_PER_GATHER = 32    # 4096 indices per dma_gather instruction
NWIRE = CHUNKS_PER_GATHER * 128 // 16  # 400

F32 = mybir.dt.float32
BF16 = mybir.dt.bfloat16
I16 = mybir.dt.int16

LAST_RESULTS = None       # test harness reads profiling info from here


# ----------------------------------------------------------------------------
# Host-side graph preprocessing
# ----------------------------------------------------------------------------

def _pack_core(nodes, degA, degB):
    """Pack `nodes` into NBLK blocks of <=128 nodes, balancing A/B edge loads.
    Returns (blocks: list[list[node]], maxA, maxB)."""
    order = np.argsort(-(degA + degB), kind="stable")
    loadA = np.zeros(NBLK, np.int64)
    loadB = np.zeros(NBLK, np.int64)
    cnt = np.zeros(NBLK, np.int64)
    blocks = [[] for _ in range(NBLK)]
    wA = 1.0 / max(1.0, degA.sum() / NBLK)   # normalize per-stream loads
    wB = 1.0 / max(1.0, degB.sum() / NBLK)
    for idx in order:
        da, db = degA[idx], degB[idx]
        score = np.maximum((loadA + da) * wA, (loadB + db) * wB)
        score[cnt >= 128] = np.inf
        b = int(np.argmin(score))
        blocks[b].append(nodes[idx])
        loadA[b] += da
        loadB[b] += db
        cnt[b] += 1
    return blocks, int(loadA.max()), int(loadB.max())


def _preprocess(edge_index, y_edge_index):
    ei = np.concatenate([np.asarray(edge_index), np.asarray(y_edge_index)], axis=1)
    src = ei[0].astype(np.int64)
    dst = ei[1].astype(np.int64)

    deg = np.bincount(dst, minlength=N).astype(np.float64) + 1.0
    dinv = (1.0 / np.sqrt(deg)).astype(np.float32)
    sqd = np.sqrt(deg).astype(np.float32)

    is_a_src = src < SPLIT // ROWS_PER_CORE * NBPC  # src node in cores 0-4

    degA_all = np.bincount(dst[is_a_src], minlength=N)
    degB_all = np.bincount(dst[~is_a_src], minlength=N)

    core_blocks = []
    maxA = maxB = 0
    for c in range(NC):
        lo, hi = c * NBPC, (c + 1) * NBPC
        nodes = np.arange(lo, hi)
        blocks, mA, mB = _pack_core(nodes, degA_all[lo:hi], degB_all[lo:hi])
        core_blocks.append(blocks)
        maxA = max(maxA, mA)
        maxB = max(maxB, mB)

    kA = max(1, -(-maxA // 128))
    kB = max(1, -(-maxB // 128))

    # device row assignment
    node_devrow = np.empty(N, np.int64)
    devrow_node = np.full(DEV_ROWS, -1, np.int64)
    for c in range(NC):
        for b, blk in enumerate(core_blocks[c]):
            base = c * ROWS_PER_CORE + b * 128
            for p, n in enumerate(blk):
                node_devrow[n] = base + p
                devrow_node[base + p] = n
    assert (node_devrow >= 0).all()
    assert ((node_devrow < SPLIT) == (np.arange(N) < 5 * NBPC)).all()

    src_dev = node_devrow[src]
    dst_dev = node_devrow[dst]

    nIA = -(-NBLK * kA // CHUNKS_PER_GATHER)
    nIB = -(-NBLK * kB // CHUNKS_PER_GATHER)

    per_core = []
    for c in range(NC):
        m = (dst >= c * NBPC) & (dst < (c + 1) * NBPC)
        es, ed = src_dev[m], dst_dev[m] - c * ROWS_PER_CORE
        blk = ed // 128
        dloc = ed % 128
        isa = es < SPLIT

        # slot-major edge layout: for each block, A-edges then B-edges
        idxA = np.zeros((nIA, CHUNKS_PER_GATHER, 128), np.int16)
        idxB = np.zeros((nIB, CHUNKS_PER_GATHER, 128), np.int16)
        nslot = NBLK * (kA + kB)
        dstloc = np.full((128, nslot), -1.0, np.float32)

        for b in range(NBLK):
            for is_a in (True, False):
                sel = (blk == b) & (isa == is_a)
                e_src = es[sel] if is_a else es[sel] - SPLIT
                e_dl = dloc[sel]
                o = np.argsort(e_src, kind="stable")  # HBM row locality
                e_src, e_dl = e_src[o], e_dl[o]
                k = kA if is_a else kB
                assert len(e_src) <= k * 128, (c, b, is_a, len(e_src))
                for t in range(len(e_src)):
                    j, lane = t // 128, t % 128
                    slot_g = b * k + j
                    gi, pos = slot_g // CHUNKS_PER_GATHER, slot_g % CHUNKS_PER_GATHER
                    if is_a:
                        idxA[gi, pos, lane] = e_src[t]
                    else:
                        idxB[gi, pos, lane] = e_src[t]
                    jj = b * (kA + kB) + (j if is_a else kA + j)
                    dstloc[lane, jj] = e_dl[t]

        # wire layout per gather: [128, 400] where [i%16 + 16r, i//16] = idx_i;
        # all gathers packed into one [128, nI*400] tensor for a single load
        def wire(a):
            nI = a.shape[0]
            flat = a.reshape(nI, CHUNKS_PER_GATHER * 128)
            w = flat.reshape(nI, -1, 16).transpose(0, 2, 1)  # [nI, 16, 400]
            w = np.tile(w, (1, 8, 1))                        # [nI, 128, 400]
            return np.ascontiguousarray(
                w.transpose(1, 0, 2).reshape(128, nI * NWIRE)).astype(np.int16)

        rows = devrow_node[c * ROWS_PER_CORE:(c + 1) * ROWS_PER_CORE]
        valid = rows >= 0
        dinv_sb = np.zeros((128, NBLK), np.float32)
        dv = np.zeros(ROWS_PER_CORE, np.float32)
        dv[valid] = dinv[rows[valid]]
        dinv_sb[:, :] = dv.reshape(NBLK, 128).T
        sq = np.zeros(ROWS_PER_CORE, np.float32)
        sq[valid] = sqd[rows[valid]]
        sqd_row = sq[None, :]

        per_core.append(dict(
            idxA=wire(idxA), idxB=wire(idxB),
            dstloc=dstloc.astype(ml_dtypes.bfloat16),
            dinv_sb=dinv_sb, sqd_row=sqd_row, rows=rows,
        ))

    return per_core, kA, kB, devrow_node, node_devrow


# ----------------------------------------------------------------------------
# Device program
# ----------------------------------------------------------------------------

def _build_program(kA, kB):
    nIA = -(-NBLK * kA // CHUNKS_PER_GATHER)
    nIB = -(-NBLK * kB // CHUNKS_PER_GATHER)
    CC = kA + kB
    NSLOT = NBLK * CC
    SELW = 16  # sel slots per DVE instruction

    nc = bacc.Bacc("TRN2", target_bir_lowering=False, debug=False,
                   num_devices=NC, num_swdge_queues=4)

    # inputs
    xT = nc.dram_tensor("xT", [128, ROWS_PER_CORE], BF16, kind="ExternalInput")
    W1b = nc.dram_tensor("W1b", [D1, D1], BF16, kind="ExternalInput")
    W2b = nc.dram_tensor("W2b", [D1, D2], BF16, kind="ExternalInput")
    b1r = nc.dram_tensor("b1r", [1, D1], F32, kind="ExternalInput")
    b2r = nc.dram_tensor("b2r", [1, D2], F32, kind="ExternalInput")
    iota = nc.dram_tensor("iota", [128, 128], BF16, kind="ExternalInput")
    identb = nc.dram_tensor("identb", [128, 128], BF16, kind="ExternalInput")
    dstloc_d = nc.dram_tensor("dstloc", [128, NSLOT], BF16, kind="ExternalInput")
    dinv_d = nc.dram_tensor("dinv_sb", [128, NBLK], F32, kind="ExternalInput")
    sqd_d = nc.dram_tensor("sqd_row", [1, ROWS_PER_CORE], F32, kind="ExternalInput")
    idxA_d = nc.dram_tensor("idxA", [128, nIA * NWIRE], I16, kind="ExternalInput")
    idxB_d = nc.dram_tensor("idxB", [128, nIB * NWIRE], I16, kind="ExternalInput")

    zcat = nc.dram_tensor("zcat", [ROWS_PER_CORE, D2], F32, kind="ExternalOutput")

    # internal DRAM.  t2_part rows are 256B padded (cols 64:128 unused).
    x1_part = nc.dram_tensor("x1_part", [ROWS_PER_CORE, D1], BF16)
    x1_full = nc.dram_tensor("x1_full", [DEV_ROWS, D1], BF16, addr_space="Shared")
    t2_part = nc.dram_tensor("t2_part", [ROWS_PER_CORE, 128], BF16)
    t2_full = nc.dram_tensor("t2_full", [DEV_ROWS, 128], BF16, addr_space="Shared")

    with tile.TileContext(nc) as tc:
        with (
            tc.tile_pool(name="const", bufs=1) as cp,
            tc.tile_pool(name="sbuf", bufs=2) as sb,
            tc.tile_pool(name="gat", bufs=7) as gp,
            tc.tile_pool(name="selp", bufs=3) as selp,
            tc.tile_pool(name="psum", bufs=2, space="PSUM") as pp,
            tc.tile_pool(name="psum3", bufs=3, space="PSUM") as pp3,
        ):
            # resident constants
            w1b_t = cp.tile([D1, D1], BF16)
            nc.sync.dma_start(w1b_t[:], W1b[:])
            w2b_t = cp.tile([D1, D2], BF16)
            nc.sync.dma_start(w2b_t[:], W2b[:])
            b1_t = cp.tile([1, D1], F32)
            nc.sync.dma_start(b1_t[:], b1r[:])
            b2_t = cp.tile([1, D2], F32)
            nc.sync.dma_start(b2_t[:], b2r[:])
            iota_t = cp.tile([128, 128], BF16)
            nc.sync.dma_start(iota_t[:], iota[:])
            idb_t = cp.tile([128, 128], BF16)
            nc.sync.dma_start(idb_t[:], identb[:])
            dstloc_t = cp.tile([128, NSLOT], BF16)
            nc.sync.dma_start(dstloc_t[:], dstloc_d[:])
            dinv_t = cp.tile([128, NBLK], F32)
            nc.sync.dma_start(dinv_t[:], dinv_d[:])
            sqd_t = cp.tile([1, ROWS_PER_CORE], F32)
            nc.sync.dma_start(sqd_t[:], sqd_d[:])
            ia_t = cp.tile([128, nIA * NWIRE], I16)
            nc.sync.dma_start(ia_t[:], idxA_d[:])
            ib_t = cp.tile([128, nIB * NWIRE], I16)
            nc.sync.dma_start(ib_t[:], idxB_d[:])
            # SBUF-resident local tables (self-loop operands)
            x1_sb = cp.tile([128, NBLK * D1], BF16)
            t2_sb = cp.tile([128, NBLK * D2], BF16)

            def batched_write(dst_dram, dst_cols, src_ap, b0, nb, w):
                """write nb blocks of [128, w] from src_ap (w-wide col groups)
                into dst_dram rows [b0*128, (b0+nb)*128) cols [0, w)."""
                dd = dst_dram[:]
                out = AP(dd.tensor, dd.offset + b0 * 128 * dst_cols,
                         [[dst_cols, 128], [128 * dst_cols, nb], [1, w]])
                src = AP(src_ap.tensor, src_ap.offset,
                         [src_ap.ap[0], [w, nb], [1, w]])
                nc.scalar.dma_start(out, src)

            # ---------------- stage A: x1 = dinv * (x @ W1) ----------------
            for b in range(NBLK):
                if b % 10 == 0:
                    xb = sb.tile([128, 1280], BF16, tag="xTb")
                    w = min(1280, (NBLK - b) * 128)
                    nc.scalar.dma_start(xb[:, :w], xT[:, b * 128:b * 128 + w])
                ps = pp3.tile([128, D1], F32, tag="agg")
                nc.tensor.matmul(ps[:], lhsT=xb[:, (b % 10) * 128:(b % 10 + 1) * 128],
                                 rhs=w1b_t[:], start=True, stop=True)
                nc.scalar.activation(x1_sb[:, b * D1:(b + 1) * D1], ps[:],
                                     mybir.ActivationFunctionType.Copy,
                                     scale=dinv_t[:, b:b + 1])
                if b % 5 == 4:
                    batched_write(x1_part, D1,
                                  x1_sb[:, (b - 4) * D1:(b + 1) * D1],
                                  b - 4, 5, D1)

            nc.gpsimd.collective_compute(
                "AllGather", mybir.AluOpType.bypass,
                replica_groups=[list(range(NC))],
                ins=[x1_part[:]], outs=[x1_full[:]],
            )

            # ---------------- generic aggregation layer --------------------
            def agg_layer(tblA, tblB, d_out, b_row, self_sb, epilogue):
                """psum_b = sqd_b x b_row + I @ self_sb[b] + sum_j sel_j^T @ g_j;
                epilogue(b, psum)."""
                gathers = []
                ia = ib = 0
                while ia < nIA or ib < nIB:
                    if ib >= nIB or (ia < nIA and ia * nIB <= ib * nIA):
                        gathers.append((True, ia)); ia += 1
                    else:
                        gathers.append((False, ib)); ib += 1

                gtiles = {}
                for n_em, (is_a, gi) in enumerate(gathers):
                    g = gp.tile([128, CHUNKS_PER_GATHER, 128], BF16,
                                tag="gA" if is_a else "gB")
                    it = (ia_t if is_a else ib_t)
                    nc.gpsimd.dma_gather(
                        out_ap=g[:], in_ap=(tblA if is_a else tblB),
                        idxs_ap=it[:, gi * NWIRE:(gi + 1) * NWIRE],
                        num_idxs=CHUNKS_PER_GATHER * 128,
                        num_idxs_reg=CHUNKS_PER_GATHER * 128,
                        elem_size=128, single_packet=False,
                        queue_num=n_em % 4,
                    )
                    gtiles[(is_a, gi)] = g

                # selection matrices, SELW slots at a time (bf16 in / bf16 out)
                stiles = {}
                for j0 in range(0, NSLOT, SELW):
                    w = min(SELW, NSLOT - j0)
                    st = selp.tile([128, SELW * 128], BF16, tag="sel")
                    base = dstloc_t[:, j0:j0 + w]
                    in0 = AP(dstloc_t.tensor, base.offset,
                             [base.ap[0], [1, w], [0, 128]])
                    it0 = iota_t[:]
                    in1 = AP(iota_t.tensor, it0.offset,
                             [it0.ap[0], [0, w], [1, 128]])
                    nc.vector.tensor_tensor(out=st[:, :w * 128], in0=in0,
                                            in1=in1,
                                            op=mybir.AluOpType.is_equal)
                    stiles[j0] = st

                for b in range(NBLK):
                    pool_ = pp3 if d_out == D1 else pp
                    ps = pool_.tile([128, d_out], F32,
                                    tag="agg" if d_out == D1 else "agg2")
                    nc.tensor.matmul(ps[:], lhsT=sqd_t[:, b * 128:(b + 1) * 128],
                                     rhs=b_row[:], start=True, stop=False)
                    nc.tensor.matmul(
                        ps[:], lhsT=idb_t[:],
                        rhs=self_sb[:, b * d_out:(b + 1) * d_out],
                        start=False, stop=False)
                    for j in range(CC):
                        is_a = j < kA
                        k = kA if is_a else kB
                        sg = b * k + (j if is_a else j - kA)
                        gi, pos = sg // CHUNKS_PER_GATHER, sg % CHUNKS_PER_GATHER
                        g = gtiles[(is_a, gi)]
                        jj = b * CC + j
                        st = stiles[(jj // SELW) * SELW]
                        off = (jj % SELW) * 128
                        nc.tensor.matmul(
                            ps[:], lhsT=st[:, off:off + 128],
                            rhs=g[:, pos, 0:d_out],
                            start=False, stop=(j == CC - 1))
                    epilogue(b, ps)

            # ---------------- layer 1 --------------------------------------
            def l1_epilogue(b, ps):
                h = sb.tile([128, D1], BF16, tag="h")
                nc.scalar.activation(h[:], ps[:],
                                     mybir.ActivationFunctionType.Relu,
                                     scale=dinv_t[:, b:b + 1])
                pt = pp.tile([128, 128], BF16, tag="tp")
                nc.tensor.transpose(pt[:], h[:], idb_t[:])
                ht = sb.tile([128, 128], BF16, tag="ht")
                nc.vector.tensor_copy(ht[:], pt[:])
                p2 = pp.tile([128, D2], F32, tag="agg2")
                nc.tensor.matmul(p2[:], lhsT=ht[:], rhs=w2b_t[:],
                                 start=True, stop=True)
                nc.scalar.activation(t2_sb[:, b * D2:(b + 1) * D2], p2[:],
                                     mybir.ActivationFunctionType.Copy,
                                     scale=dinv_t[:, b:b + 1])
                if b % 5 == 4:
                    batched_write(t2_part, 128,
                                  t2_sb[:, (b - 4) * D2:(b + 1) * D2],
                                  b - 4, 5, D2)

            agg_layer(x1_full[0:SPLIT, :], x1_full[SPLIT:DEV_ROWS, :], D1,
                      b1_t, x1_sb, l1_epilogue)

            nc.gpsimd.collective_compute(
                "AllGather", mybir.AluOpType.bypass,
                replica_groups=[list(range(NC))],
                ins=[t2_part[:]], outs=[t2_full[:]],
            )

            # ---------------- layer 2 --------------------------------------
            z_sb = cp.tile([128, 5 * D2], F32)

            def l2_epilogue(b, ps):
                nc.scalar.activation(z_sb[:, (b % 5) * D2:(b % 5 + 1) * D2],
                                     ps[:],
                                     mybir.ActivationFunctionType.Copy,
                                     scale=dinv_t[:, b:b + 1])
                if b % 5 == 4:
                    batched_write(zcat, D2, z_sb[:], b - 4, 5, D2)

            agg_layer(t2_full[0:SPLIT, :], t2_full[SPLIT:DEV_ROWS, :], D2,
                      b2_t, t2_sb, l2_epilogue)

    nc.compile()
    return nc


# ----------------------------------------------------------------------------

def kernel(x, edge_index, y_edge_index, W1, b1, W_mu, b_mu, W_ls, b_ls):
    global LAST_RESULTS
    try:  # enable NTFF profiling under axon when available (no-op otherwise)
        from trn_agent_boot.trn_boot import _ntff_profile_via_ctypes
        try:
            from antenv import axon_hooks
        except ImportError:
            import types
            import antenv
            axon_hooks = types.ModuleType('antenv.axon_hooks')
            axon_hooks._hook = None
            def _set(h):
                axon_hooks._hook = h
            def _get():
                return axon_hooks._hook
            axon_hooks.set_axon_ntff_profile_hook = _set
            axon_hooks.get_axon_ntff_profile_hook = _get
            sys.modules['antenv.axon_hooks'] = axon_hooks
            antenv.axon_hooks = axon_hooks
        if axon_hooks.get_axon_ntff_profile_hook() is None:
            axon_hooks.set_axon_ntff_profile_hook(
                _ntff_profile_via_ctypes('/opt/axon/libaxon_pjrt.so'))
    except Exception:
        pass

    x = np.asarray(x, np.float32)
    W1 = np.asarray(W1, np.float32)
    b1 = np.asarray(b1, np.float32)
    W2 = np.concatenate([np.asarray(W_mu, np.float32),
                         np.asarray(W_ls, np.float32)], axis=1)
    b2 = np.concatenate([np.asarray(b_mu, np.float32),
                         np.asarray(b_ls, np.float32)])

    per_core, kA, kB, devrow_node, node_devrow = _preprocess(
        edge_index, y_edge_index)

    nc = _build_program(kA, kB)

    iota_np = np.tile(np.arange(128, dtype=np.float32)[None, :], (128, 1))
    ident_np = np.eye(128, dtype=np.float32)

    in_maps = []
    for c in range(NC):
        pc = per_core[c]
        rows = pc["rows"]
        xTc = np.zeros((128, ROWS_PER_CORE), np.float32)
        valid = rows >= 0
        xTc[:, valid] = x[rows[valid]].T
        in_maps.append(dict(
            xT=xTc.astype(ml_dtypes.bfloat16),
            W1b=W1.astype(ml_dtypes.bfloat16),
            W2b=W2.astype(ml_dtypes.bfloat16),
            b1r=b1[None, :], b2r=b2[None, :],
            iota=iota_np.astype(ml_dtypes.bfloat16),
            identb=ident_np.astype(ml_dtypes.bfloat16),
            dstloc=pc["dstloc"], dinv_sb=pc["dinv_sb"], sqd_row=pc["sqd_row"],
            idxA=pc["idxA"], idxB=pc["idxB"],
        ))

    res = run_bass_kernel_spmd(nc, in_maps, list(range(NC)))
    LAST_RESULTS = res

    z_dev = np.concatenate([res.results[c]["zcat"] for c in range(NC)], axis=0)
    z = z_dev[node_devrow]  # [N, 64]
    return z[:, :32].astype(np.float32), z[:, 32:].astype(np.float32)

